# revision 10
# baseline (speedup 1.0000x reference)
"""Trainium2 Bass kernel for nn_CNNFeatMoe (CNN feature MoE with top-2 routing).

V1 redesign vs baseline:
  - pooling pipeline rebalanced across ACT/DVE/Pool(gpsimd):
      ACT drains psum-even (relu+bias), DVE psum-odd (stt bias+max),
      h-max second stage on Pool/DVE (tunable), some conv1 tiles use
      double-ACT drain + DVE-4x sbuf w-max ("D-scheme") to offload DVE.
  - gate conv1 pair-packed: 2 samples per psum tile via PE column tiling.
  - conv2 im2col frames built as ONE tile F3 [96=(dx,ch), 4=expert, 34*34]
    -> 3 SBUF DMAs per sample instead of 12 (conv1 expert weight columns
    reordered host-side to (ch, expert)).
  - pooled gate/conv2 outputs written DIRECTLY into the FC-layout tensors
    (gpoolP / h2P) by the h-max ops; gfw/efw reordered host-side to match.
  - FC phase: PE column-tiling x4 (4 concurrent K-chunks), deep weight
    prefetch on two HWDGE queues (SP + ACT).
  - softmax tail: pairwise max/sum instead of tensor_reduce.
"""

import numpy as np
import ml_dtypes
from contextlib import ExitStack

import concourse.bass as bass
import concourse.mybir as mybir
import concourse.tile as tile
from concourse.vector_clock import ScopedClock
from concourse.masks import make_identity
import bass_rust

F32 = mybir.dt.float32
BF16 = mybir.dt.bfloat16
AX = mybir.AxisListType
ALU = mybir.AluOpType
ACTF = mybir.ActivationFunctionType

N_CORES = 8
B = 256
S = B // N_CORES          # samples per core
C_IN, H, W = 3, 64, 64
N_EXPERTS, TOP_K, C_OUT = 4, 2, 64
HP, WP = H + 2, W + 2     # 66x66 zero-padded frame
FRAME = 34                # padded 32x32 frame rows for conv2 input
XRF = H * W               # 4096: pre-shifted replica, contiguous 64x64

# tunables (engine assignment / scheme mix)
N_DSCHEME = 4             # of 8 conv1-expert blocks per sample use double-ACT drain

bf16 = ml_dtypes.bfloat16


# --------------------------------------------------------------------------
# Walrus in this environment accepts at most ONE sync wait per instruction.
# Split extra waits onto same-engine NoOps inserted right before.
# --------------------------------------------------------------------------
def _legalize_single_wait(nc):
    for _name, bbb in nc.bb_map.items():
        bb = bbb.bb if hasattr(bbb, "bb") else bbb
        insts = bb.instructions
        i = 0
        while i < len(insts):
            inst = insts[i]
            si = inst.sync_info
            if si is not None and len(si.on_wait) > 1:
                waits = list(si.on_wait)
                si.on_wait = [waits[-1]]
                for w in waits[:-1]:
                    nop = bass_rust.InstNoOp(
                        name=nc.get_next_instruction_name(), engine=inst.engine
                    )
                    nop.sync_info = mybir.SyncInfo(on_wait=[w], on_update=[])
                    nc.register_instruction(nop)
                    insts.insert(i, nop)
                    i += 1
            i += 1


class SplitWaitTileContext(tile.TileContext):
    def _drain_and_barrier(self, tick_clock, wait_clock):
        drain_inst = self.nc.sync.drain()
        wait_clock.add_sem_waits(
            drain_inst.ins, ScopedClock({None: tick_clock.global_clock})
        )
        self.nc.all_engine_barrier()
        assert self.sems is not None
        popped = self.nc._tile_sem_poison_stack.pop()
        assert popped is self._sem_poison
        self.nc.clear_and_free_semaphores(list(self.sems.allocated().values()))
        self.nc.all_engine_barrier()
        _legalize_single_wait(self.nc)


# --------------------------------------------------------------------------
# Device program
# --------------------------------------------------------------------------
def _emit(nc, tc, ctx, s_per_core, loop_n=1):
    Sn = s_per_core
    xr_d = nc.dram_tensor("xr", [Sn, 81, XRF], BF16, kind="ExternalInput")
    w1_d = nc.dram_tensor("w1", [81, 192], BF16, kind="ExternalInput")
    bg2_d = nc.dram_tensor("bg2", [128, 1], F32, kind="ExternalInput")
    b1e_d = nc.dram_tensor("b1e", [128, 1], F32, kind="ExternalInput")
    w2_d = nc.dram_tensor("w2", [96, 4, 3, 64], BF16, kind="ExternalInput")
    b2_d = nc.dram_tensor("b2", [128, 2], F32, kind="ExternalInput")
    efwp_d = nc.dram_tensor("efwp", [4, 128, 128, 128], BF16, kind="ExternalInput")
    efb_d = nc.dram_tensor("efb", [1, 4, 128], BF16, kind="ExternalInput")
    gfb_d = nc.dram_tensor("gfb", [1, 4], F32, kind="ExternalInput")
    gfwp_d = nc.dram_tensor("gfwp", [128, 512, 4], F32, kind="ExternalInput")
    fwt_d = nc.dram_tensor("fwt", [128, 10], F32, kind="ExternalInput")
    fb_d = nc.dram_tensor("fb", [10, 1], F32, kind="ExternalInput")
    out_d = nc.dram_tensor("out", [10, Sn], F32, kind="ExternalOutput")

    singles = ctx.enter_context(tc.tile_pool(name="singles", bufs=1))
    persist = ctx.enter_context(tc.tile_pool(name="persist", bufs=1))
    stage = ctx.enter_context(tc.tile_pool(name="stage", bufs=2))
    evpool = ctx.enter_context(tc.tile_pool(name="evpool", bufs=5))
    wpool = ctx.enter_context(tc.tile_pool(name="wpool", bufs=4))
    wmgpool = ctx.enter_context(tc.tile_pool(name="wmgpool", bufs=3))
    psE = ctx.enter_context(tc.tile_pool(name="psE", bufs=5, space="PSUM"))
    ps2 = ctx.enter_context(tc.tile_pool(name="ps2", bufs=2, space="PSUM"))
    psf = ctx.enter_context(tc.tile_pool(name="psf", bufs=1, space="PSUM"))
    efpool = ctx.enter_context(tc.tile_pool(name="efpool", bufs=3))

    # ---- weights / constants to SBUF ----
    w1_sb = singles.tile([81, 192], BF16)
    nc.sync.dma_start(out=w1_sb, in_=w1_d[:])
    bg2_sb = singles.tile([128, 1], F32)
    nc.sync.dma_start(out=bg2_sb, in_=bg2_d[:])
    b1e_sb = singles.tile([128, 1], F32)
    nc.sync.dma_start(out=b1e_sb, in_=b1e_d[:])
    w2_sb = singles.tile([96, 4, 3, 64], BF16)
    nc.sync.dma_start(out=w2_sb, in_=w2_d[:])
    b2_sb = singles.tile([128, 2], F32)
    nc.sync.dma_start(out=b2_sb, in_=b2_d[:])
    gfwp_sb = singles.tile([128, 512, 4], F32)
    nc.sync.dma_start(out=gfwp_sb, in_=gfwp_d[:])
    fwt_sb = singles.tile([128, 10], F32)
    nc.sync.dma_start(out=fwt_sb, in_=fwt_d[:])
    fb_sb = singles.tile([10, 1], F32)
    nc.sync.dma_start(out=fb_sb, in_=fb_d[:])
    efb_sb = singles.tile([1, 4, 128], BF16)
    nc.sync.dma_start(out=efb_sb, in_=efb_d[:])
    gfb_sb = singles.tile([1, 4], F32)
    nc.sync.dma_start(out=gfb_sb, in_=gfb_d[:])
    ident = singles.tile([128, 128], F32)
    make_identity(nc, ident)
    ones_bf = singles.tile([1, Sn], BF16)
    nc.vector.memset(ones_bf, 1.0)
    ones_f = singles.tile([1, Sn], F32)
    nc.vector.memset(ones_f, 1.0)

    # ---- persistent activations ----
    # gpoolP[(jbit,ch), s, j]: pooled gate features, j = b*128+hp*32+wp (b<4)
    gpoolP = persist.tile([128, Sn, 512], F32, tag="gpoolP")
    # h2P[(jbit,o2), e, s, j]: pooled conv2 features, j = (hp%8)*16+wp
    h2P = persist.tile([128, 4, Sn, 128], BF16, tag="h2P")
    xr_sb = [persist.tile([81, XRF], BF16, tag=f"xr{i}", name=f"xr{i}") for i in range(2)]
    # fr: conv1 expert pooled frames, partitions (o32, e4); zero borders once
    fr_sb = [persist.tile([128, FRAME * FRAME], BF16, tag=f"fr{i}", name=f"frames{i}") for i in range(2)]
    for fr in fr_sb:
        f34 = fr.rearrange("p (h w) -> p h w", h=FRAME)
        nc.vector.memset(f34[:, 0, :], 0.0)
        nc.vector.memset(f34[:, 33, :], 0.0)
        nc.vector.memset(f34[:, 1:33, 0], 0.0)
        nc.vector.memset(f34[:, 1:33, 33], 0.0)
    # F3[(dx,o32), e, h, 32]: dx-shifted replicas of fr, contiguous 32-wide rows
    F3_sb = [persist.tile([96, 4, FRAME, 32], BF16, tag=f"F3{i}", name=f"F3{i}")
             for i in range(2)]


    # ---- conv phase (optionally repeated on-device for timing) ----
    loop_cm = tc.For_i(0, loop_n, 1) if loop_n > 1 else None
    if loop_cm is not None:
        ctx.enter_context(loop_cm)

    for s in range(Sn):
        xr = xr_sb[s % 2]
        fr = fr_sb[s % 2]
        nc.sync.dma_start(out=xr, in_=xr_d[s])
        xr66 = xr.rearrange("p (h w) -> p h w", h=H)
        f34 = fr.rearrange("p (h w) -> p h w", h=FRAME)

        # ---------- conv1 expert + gate blocks interleaved ----------
        def emit_gate_block(gb):
            xr_prev = xr_sb[(s - 1) % 2]
            xr66p = xr_prev.rearrange("p (h w) -> p h w", h=H)
            wmg = wmgpool.tile([128, 16, 32], F32, tag="wmg")
            for half in range(2):
                pb2 = gb * 2 + half
                h0g = pb2 * 8
                pg = psE.tile([128, 8, 64], F32, tag="psE", name=f"pg{s}_{pb2}")
                nc.tensor.matmul(pg[0:64], w1_sb[:, 0:64],
                                 xr66p[:, h0g : h0g + 8, :],
                                 start=True, stop=True, tile_position=(0, 0))
                nc.tensor.matmul(pg[64:128], w1_sb[:, 0:64],
                                 xr66[:, h0g : h0g + 8, :],
                                 start=True, stop=True, tile_position=(0, 64))
                evg = evpool.tile([128, 8, 32], F32, tag="evg")
                nc.scalar.activation(evg, pg[:, :, 0:64:2], ACTF.Relu,
                                     bias=bg2_sb)
                nc.vector.scalar_tensor_tensor(
                    out=wmg[:, 8 * half : 8 * half + 8, :],
                    in0=pg[:, :, 1:64:2], scalar=bg2_sb, in1=evg,
                    op0=ALU.add, op1=ALU.max)
            jbit = gb // 2
            jcol = (gb % 2) * 256
            for sh in range(2):
                samp = s - 1 + sh
                dst = gpoolP[64 * jbit : 64 * jbit + 64, samp,
                             jcol : jcol + 256].rearrange(
                                 "p (a b) -> p a b", a=8)
                nc.vector.tensor_tensor(
                    out=dst,
                    in0=wmg[64 * sh : 64 * sh + 64, 0:16:2, :],
                    in1=wmg[64 * sh : 64 * sh + 64, 1:16:2, :],
                    op=ALU.max)

        for pb in range(8):
            h0 = pb * 8
            pe = psE.tile([128, 8, 64], F32, tag="psE")
            rhs = xr66[:, h0 : h0 + 8, :]                    # [81, 8, 64] contiguous
            nc.tensor.matmul(pe, w1_sb[:, 64:192], rhs, start=True, stop=True)
            wm1 = wpool.tile([128, 8, 32], BF16, tag="wm1")
            if pb < N_DSCHEME:
                # D-scheme: double ACT drain, w-max on DVE 4x (sbuf bf16)
                ev = evpool.tile([128, 8, 32], BF16, tag="ev")
                nc.scalar.activation(ev, pe[:, :, 0:64:2], ACTF.Relu, bias=b1e_sb)
                od = evpool.tile([128, 8, 32], BF16, tag="od")
                nc.scalar.activation(od, pe[:, :, 1:64:2], ACTF.Relu, bias=b1e_sb)
                nc.vector.tensor_tensor(out=wm1, in0=ev, in1=od, op=ALU.max)
            else:
                # S-scheme: ACT even + DVE odd-stt
                ev = evpool.tile([128, 8, 32], BF16, tag="ev")
                nc.scalar.activation(ev, pe[:, :, 0:64:2], ACTF.Relu, bias=b1e_sb)
                nc.vector.scalar_tensor_tensor(out=wm1, in0=pe[:, :, 1:64:2],
                                               scalar=b1e_sb, in1=ev,
                                               op0=ALU.add, op1=ALU.max)
            # h-max into frame interior rows (Pool engine)
            hp0 = pb * 4
            nc.vector.tensor_tensor(out=f34[:, 1 + hp0 : 5 + hp0, 1:33],
                                    in0=wm1[:, 0:8:2, :], in1=wm1[:, 1:8:2, :],
                                    op=ALU.max)
            if s % 2 == 1 and pb % 2 == 1:
                emit_gate_block(pb // 2)

        # ---------- F3 build: 3 dx-shift replication DMAs (row-major, 32-wide) ----------
        F3 = F3_sb[s % 2]
        for g in range(3):
            nc.sync.dma_start(
                out=F3[32 * g : 32 * g + 32],
                in_=f34[:, :, g : g + 32],
            )
        F3v = F3

        # ---------- conv2: per expert pair, 2 psum tiles (rh halves) ----------
        for pair in range(2):
            wm2 = wpool.tile([128, 32, 16], BF16, tag="wm2")
            for rh in range(2):
                p2 = ps2.tile([128, 16, 32], F32, tag="ps2")
                for dy in range(3):
                    for side in range(2):
                        e = 2 * pair + side
                        rhs = F3v[:, e, dy + rh * 16 : dy + rh * 16 + 16, :]
                        nc.tensor.matmul(
                            p2[64 * side : 64 * side + 64],
                            w2_sb[:, e, dy, :], rhs,
                            start=(dy == 0), stop=(dy == 2),
                            tile_position=(0, 64 * side),
                        )
                ev2 = evpool.tile([128, 16, 16], BF16, tag="ev2")
                nc.scalar.activation(ev2, p2[:, :, 0:32:2], ACTF.Relu,
                                     bias=b2_sb[:, pair : pair + 1])
                nc.vector.scalar_tensor_tensor(
                    out=wm2[:, 16 * rh : 16 * rh + 16, :],
                    in0=p2[:, :, 1:32:2],
                    scalar=b2_sb[:, pair : pair + 1],
                    in1=ev2, op0=ALU.add, op1=ALU.max)
            # h-max + remap into h2P (DVE 4x: all bf16 sbuf)
            for side in range(2):
                e = 2 * pair + side
                for jbit in range(2):
                    dst = h2P[64 * jbit : 64 * jbit + 64, e, s, :].rearrange(
                        "p (a b) -> p a b", a=8)
                    nc.vector.tensor_tensor(
                        out=dst,
                        in0=wm2[64 * side : 64 * side + 64,
                                16 * jbit : 16 * jbit + 16 : 2, :],
                        in1=wm2[64 * side : 64 * side + 64,
                                16 * jbit + 1 : 16 * jbit + 16 : 2, :],
                        op=ALU.max)

    # ---- FC phase ----
    # gate FC: col-tiled x4, fp32
    psG4 = psf.tile([128, 4], F32, tag="acc", name="psG4")
    for j in range(512):
        g = j % 4
        nc.tensor.matmul(psG4[32 * g : 32 * g + 32, :],
                         gpoolP[:, :, j], gfwp_sb[:, j, :],
                         start=(j < 4), stop=(j >= 508 and g != 0),
                         tile_position=(0, 32 * g))
    nc.tensor.matmul(psG4[0:32, :], ones_f, gfb_sb,
                     start=False, stop=True, tile_position=(0, 0))
    gacc = stage.tile([Sn, 2, 4], F32, tag="gacc", name="gacc")
    nc.scalar.activation(gacc[:, 0], psG4[32:64], ACTF.Identity)
    nc.scalar.activation(gacc[:, 1], psG4[96:128], ACTF.Identity)
    g1 = stage.tile([Sn, 4], F32, tag="g1")
    nc.vector.tensor_tensor(out=g1, in0=psG4[0:32], in1=gacc[:, 0], op=ALU.add)
    g2 = stage.tile([Sn, 4], F32, tag="g2")
    nc.vector.tensor_tensor(out=g2, in0=psG4[64:96], in1=gacc[:, 1], op=ALU.add)
    gs = stage.tile([Sn, 4], F32, tag="gs")
    nc.vector.tensor_tensor(out=gs, in0=g1, in1=g2, op=ALU.add)

    # expert FC: col-tiled x4; psF4[(g,s), o] accumulates group-g partials
    feats = persist.tile([Sn, 4, 128], F32, tag="feats")
    acc_sb = stage.tile([Sn, 2, 128], F32, tag="acc_sb", name="acc_sb")
    for e in range(4):
        psF4 = psf.tile([128, 128], F32, tag="acc", name=f"psF4_{e}")
        for jb in range(4):
            blk = efpool.tile([128, 32, 128], BF16, tag="efblk",
                              name=f"ef{e}_{jb}")
            eng = nc.sync if (e * 4 + jb) % 2 == 0 else nc.scalar
            eng.dma_start(out=blk, in_=efwp_d[e, :, jb * 32 : jb * 32 + 32, :])
            for jg in range(8):
                for g in range(4):
                    j = jb * 32 + jg * 4 + g
                    nc.tensor.matmul(psF4[32 * g : 32 * g + 32, :],
                                     h2P[:, e, :, j], blk[:, jg * 4 + g, :],
                                     start=(jb == 0 and jg == 0),
                                     stop=(jb == 3 and jg == 7 and g != 0),
                                     tile_position=(0, 32 * g))
        nc.tensor.matmul(psF4[0:32, :], ones_bf, efb_sb[:, e, :],
                         start=False, stop=True, tile_position=(0, 0))
        # reduce the 4 col-groups: ACT copies 2 groups to SBUF, DVE adds
        nc.scalar.activation(acc_sb[:, 0], psF4[32:64], ACTF.Identity)
        nc.scalar.activation(acc_sb[:, 1], psF4[96:128], ACTF.Identity)
        a1 = stage.tile([Sn, 128], F32, tag="a1")
        nc.vector.tensor_tensor(out=a1, in0=psF4[0:32], in1=acc_sb[:, 0],
                                op=ALU.add)
        a2 = stage.tile([Sn, 128], F32, tag="a2")
        nc.vector.tensor_tensor(out=a2, in0=psF4[64:96], in1=acc_sb[:, 1],
                                op=ALU.add)
        nc.vector.tensor_tensor(out=feats[:, e], in0=a1, in1=a2, op=ALU.add)

    # ---- softmax / top-2 / combine / head ----
    m01 = stage.tile([Sn, 1], F32, tag="m01")
    nc.vector.tensor_tensor(out=m01, in0=gs[:, 0:1], in1=gs[:, 1:2], op=ALU.max)
    m23 = stage.tile([Sn, 1], F32, tag="m23")
    nc.vector.tensor_tensor(out=m23, in0=gs[:, 2:3], in1=gs[:, 3:4], op=ALU.max)
    mx = stage.tile([Sn, 1], F32, tag="mx")
    nc.vector.tensor_tensor(out=mx, in0=m01, in1=m23, op=ALU.max)
    nmx = stage.tile([Sn, 1], F32, tag="nmx")
    nc.scalar.mul(nmx, mx, -1.0)
    ex = stage.tile([Sn, 4], F32, tag="ex")
    nc.scalar.activation(ex, gs, ACTF.Exp, bias=nmx)
    cnt = stage.tile([Sn, 4], F32, tag="cnt")
    nc.vector.memset(cnt, 0.0)
    tgt = stage.tile([Sn, 4], F32, tag="tgt")
    for ep in range(4):
        col = gs[:, ep : ep + 1].broadcast_to((Sn, 4))
        nc.vector.tensor_tensor(out=tgt, in0=col, in1=gs, op=ALU.is_gt)
        nc.vector.tensor_tensor(out=cnt, in0=cnt, in1=tgt, op=ALU.add)
        if ep < 3:
            ncols = 4 - (ep + 1)
            coleq = gs[:, ep : ep + 1].broadcast_to((Sn, ncols))
            teq = stage.tile([Sn, 4], F32, tag="teq")
            nc.vector.tensor_tensor(out=teq[:, : ncols], in0=coleq,
                                    in1=gs[:, ep + 1 :], op=ALU.is_equal)
            nc.vector.tensor_tensor(out=cnt[:, ep + 1 :], in0=cnt[:, ep + 1 :],
                                    in1=teq[:, : ncols], op=ALU.add)
    mask = stage.tile([Sn, 4], F32, tag="mask")
    nc.vector.tensor_scalar(out=mask, in0=cnt, scalar1=float(TOP_K), scalar2=None,
                            op0=ALU.is_lt)
    wm = stage.tile([Sn, 4], F32, tag="wm")
    nc.vector.tensor_tensor(out=wm, in0=ex, in1=mask, op=ALU.mult)
    s01 = stage.tile([Sn, 1], F32, tag="s01")
    nc.vector.tensor_tensor(out=s01, in0=wm[:, 0:1], in1=wm[:, 1:2], op=ALU.add)
    s23 = stage.tile([Sn, 1], F32, tag="s23")
    nc.vector.tensor_tensor(out=s23, in0=wm[:, 2:3], in1=wm[:, 3:4], op=ALU.add)
    sw = stage.tile([Sn, 1], F32, tag="sw")
    nc.vector.tensor_tensor(out=sw, in0=s01, in1=s23, op=ALU.add)
    rw = stage.tile([Sn, 1], F32, tag="rw")
    nc.vector.reciprocal(rw, sw)
    wn = stage.tile([Sn, 4], F32, tag="wn")
    nc.vector.tensor_scalar(out=wn, in0=wm, scalar1=rw, scalar2=None, op0=ALU.mult)

    feat = stage.tile([Sn, 128], F32, tag="feat")
    nc.vector.tensor_scalar(out=feat, in0=feats[:, 0], scalar1=wn[:, 0:1],
                            scalar2=None, op0=ALU.mult)
    for e in range(1, 4):
        nc.vector.scalar_tensor_tensor(out=feat, in0=feats[:, e],
                                       scalar=wn[:, e : e + 1], in1=feat,
                                       op0=ALU.mult, op1=ALU.add)
    # head: out.T = fw @ feat.T   (bias per-partition on the 10 outputs)
    psT = psf.tile([128, Sn], F32, tag="acc", name="psT")
    nc.tensor.transpose(psT, feat, ident[0:Sn, 0:Sn])
    featT = stage.tile([128, Sn], F32, tag="featT")
    nc.vector.tensor_copy(out=featT, in_=psT)
    psH = psf.tile([10, Sn], F32, tag="acc", name="psH")
    nc.tensor.matmul(psH, fwt_sb, featT, start=True, stop=True)
    outT = stage.tile([10, Sn], F32, tag="outT")
    nc.scalar.activation(outT, psH, ACTF.Identity, bias=fb_sb)
    nc.sync.dma_start(out=out_d[:], in_=outT)


def build_nc(s_per_core=S, loop_n=1):
    nc = bass.Bass()
    with ExitStack() as ctx:
        tc = ctx.enter_context(SplitWaitTileContext(nc))
        _emit(nc, tc, ctx, s_per_core, loop_n=loop_n)
    return nc


# --------------------------------------------------------------------------
# Host-side input preparation
# --------------------------------------------------------------------------
def _split_hi_lo(a):
    hi = a.astype(bf16)
    lo = (a - hi.astype(np.float32)).astype(bf16)
    return hi, lo


def prep_inputs(x, ew1, eb1, ew2, eb2, efw, efb, gw, gb, gfw, gfb, fw, fb):
    # x -> padded 66x66 frames, then 81-row pre-shifted 64x64 im2col replicas
    # (each replica row stores its own (dy,dx)-shifted window -> contiguous rhs)
    xp = np.zeros((B, C_IN, HP, HP), np.float32)
    xp[:, :, 1:65, 1:65] = x
    x_hi, x_lo = _split_hi_lo(xp)
    xr = np.empty((B, 81, XRF), bf16)
    r = 0
    for src in (x_hi, x_hi, x_lo):
        for c in range(C_IN):
            for dy in range(3):
                for dx in range(3):
                    xr[:, r] = src[:, c, dy : dy + H, dx : dx + W].reshape(B, XRF)
                    r += 1
    assert r == 81

    # conv1 fused weights: [81, 192] = 3 K-terms x (64 gate | 128 expert(o,e))
    wg = gw.transpose(1, 2, 3, 0).reshape(27, 64)            # (c,dy,dx) x o
    we = ew1.transpose(2, 3, 4, 1, 0).reshape(27, 128)       # (c,dy,dx) x (o,e)
    w_full = np.concatenate([wg, we], axis=1).astype(np.float32)
    w_hi, w_lo = _split_hi_lo(w_full)
    w1 = np.concatenate([w_hi, w_lo, w_hi], axis=0)          # [81, 192]

    bg2 = np.concatenate([gb, gb]).reshape(128, 1).astype(np.float32)
    b1e = np.ascontiguousarray(eb1.T.reshape(128, 1).astype(np.float32))  # (o,e)

    # conv2 K=96 im2col weights: w2[g*32+i, e, dy, o] = ew2[e, o, i, dy, g]
    w2 = np.ascontiguousarray(
        ew2.transpose(4, 2, 0, 3, 1).reshape(96, 4, 3, 64).astype(bf16))
    # bias per (side, ch) x pair: b2[side*64+c, pair] = eb2[2*pair+side, c]
    b2 = np.ascontiguousarray(
        eb2.reshape(2, 2, 64).transpose(1, 2, 0).reshape(128, 2)
        .astype(np.float32))

    # expert FC weights: h2P[(jbit, o2), e, s, j] = pooled[o2, 8*jbit + j//16,
    # j%16]  ->  flat = o2*256 + jbit*128 + j
    efwp = np.ascontiguousarray(
        efw.reshape(4, 128, 64, 2, 128).transpose(0, 3, 2, 4, 1)
        .reshape(4, 128, 128, 128).astype(bf16))

    # gate FC weights: gpoolP[(jbit, ch), s, jcol] -> flat = ch*1024 + jbit*512 + jcol
    gfwp = np.ascontiguousarray(
        gfw.reshape(4, 64, 2, 512).transpose(2, 1, 3, 0)
        .reshape(128, 512, 4).astype(np.float32))

    fwt = fw.T.astype(np.float32).copy()                     # [128, 10]
    fbv = fb.reshape(10, 1).astype(np.float32)

    shared = {
        "w1": np.ascontiguousarray(w1.astype(bf16)),
        "bg2": bg2, "b1e": b1e,
        "w2": np.ascontiguousarray(w2), "b2": np.ascontiguousarray(b2),
        "efwp": efwp, "gfwp": gfwp, "fwt": fwt, "fb": fbv,
        "efb": np.ascontiguousarray(efb.reshape(1, 4, 128).astype(bf16)),
        "gfb": np.ascontiguousarray(gfb.reshape(1, 4).astype(np.float32)),
    }
    return xr, shared


# --------------------------------------------------------------------------
# Persistent runner (same as baseline)
# --------------------------------------------------------------------------
class _Runner:
    def __init__(self, nc, n_cores):
        import jax
        from jax.experimental.shard_map import shard_map
        from jax.sharding import Mesh, PartitionSpec, NamedSharding
        from concourse import bass2jax

        bass2jax.install_neuronx_cc_hook()
        self.jax = jax
        self.nc = nc
        self.n_cores = n_cores
        partition_name = (
            nc.partition_id_tensor.name if nc.partition_id_tensor else None
        )
        in_names, out_names, out_avals, zero_outs = [], [], [], []
        for alloc in nc.m.functions[0].allocations:
            if not isinstance(alloc, mybir.MemoryLocationSet):
                continue
            name = alloc.memorylocations[0].name
            if alloc.kind == "ExternalInput":
                if name == partition_name:
                    continue
                in_names.append(name)
            elif alloc.kind == "ExternalOutput":
                out_names.append(name)
                shape = tuple(alloc.tensor_shape)
                dtype = mybir.dt.np(alloc.dtype)
                out_avals.append(jax.core.ShapedArray(shape, dtype))
                zero_outs.append(np.zeros(shape, dtype))
        self.in_names = list(in_names)
        self.out_names = out_names
        self.zero_outs = zero_outs
        n_params = len(in_names)
        all_names = in_names + out_names
        if partition_name is not None:
            all_names = all_names + [partition_name]
        donate = tuple(range(n_params, n_params + len(out_names)))
        out_avals_t = tuple(out_avals)

        def _body(*args):
            operands = list(args)
            if partition_name is not None:
                operands.append(bass2jax.partition_id_tensor())
            outs = bass2jax._bass_exec_p.bind(
                *operands,
                out_avals=out_avals_t,
                in_names=tuple(all_names),
                out_names=tuple(out_names),
                lowering_input_output_aliases=(),
                sim_require_finite=True,
                sim_require_nnan=True,
                nc=nc,
            )
            return tuple(outs)

        devices = jax.devices()[:n_cores]
        self.mesh = Mesh(np.asarray(devices), ("core",))
        self.sharding = NamedSharding(self.mesh, PartitionSpec("core"))
        in_specs = (PartitionSpec("core"),) * (n_params + len(out_names))
        out_specs = (PartitionSpec("core"),) * len(out_names)
        self.fn = jax.jit(
            shard_map(_body, mesh=self.mesh, in_specs=in_specs,
                      out_specs=out_specs, check_rep=False),
            donate_argnums=donate, keep_unused=True,
        )

    def concat_inputs(self, in_maps):
        return [
            np.concatenate([np.asarray(m[name]) for m in in_maps], axis=0)
            for name in self.in_names
        ]

    def put(self, concat_in):
        return [self.jax.device_put(a, self.sharding) for a in concat_in]

    def call(self, device_in):
        zeros = [np.zeros((self.n_cores * z.shape[0], *z.shape[1:]), z.dtype)
                 for z in self.zero_outs]
        outs = self.fn(*device_in, *zeros)
        return outs

    def run(self, in_maps):
        outs = self.call(self.put(self.concat_inputs(in_maps)))
        n = self.n_cores
        res = []
        for c in range(n):
            d = {}
            for i, name in enumerate(self.out_names):
                full = np.asarray(outs[i])
                d[name] = full.reshape(n, full.shape[0] // n, *full.shape[1:])[c]
            res.append(d)
        return res


_RUNNER = None


def get_runner():
    global _RUNNER
    if _RUNNER is None:
        _RUNNER = _Runner(build_nc(S), N_CORES)
    return _RUNNER


def kernel(**inputs):
    xr, shared = prep_inputs(**{k: np.asarray(v) for k, v in inputs.items()})
    runner = get_runner()
    in_maps = []
    for c in range(N_CORES):
        m = dict(shared)
        m["xr"] = np.ascontiguousarray(xr[c * S : (c + 1) * S])
        in_maps.append(m)
    res = runner.run(in_maps)
    out = np.empty((B, 10), np.float32)
    for c in range(N_CORES):
        out[c * S : (c + 1) * S] = res[c]["out"].T
    return out



# revision 33
# speedup vs baseline: 1.8070x; 1.8070x over previous
"""Trainium2 Bass kernel for nn_CNNFeatMoe (CNN feature MoE with top-2 routing).

V1 redesign vs baseline:
  - pooling pipeline rebalanced across ACT/DVE/Pool(gpsimd):
      ACT drains psum-even (relu+bias), DVE psum-odd (stt bias+max),
      h-max second stage on Pool/DVE (tunable), some conv1 tiles use
      double-ACT drain + DVE-4x sbuf w-max ("D-scheme") to offload DVE.
  - gate conv1 pair-packed: 2 samples per psum tile via PE column tiling.
  - conv2 im2col frames built as ONE tile F3 [96=(dx,ch), 4=expert, 34*34]
    -> 3 SBUF DMAs per sample instead of 12 (conv1 expert weight columns
    reordered host-side to (ch, expert)).
  - pooled gate/conv2 outputs written DIRECTLY into the FC-layout tensors
    (gpoolP / h2P) by the h-max ops; gfw/efw reordered host-side to match.
  - FC phase: PE column-tiling x4 (4 concurrent K-chunks), deep weight
    prefetch on two HWDGE queues (SP + ACT).
  - softmax tail: pairwise max/sum instead of tensor_reduce.
"""

import numpy as np
import ml_dtypes
from contextlib import ExitStack

import concourse.bass as bass
import concourse.mybir as mybir
import concourse.tile as tile
from concourse.vector_clock import ScopedClock
from concourse.masks import make_identity
import bass_rust

F32 = mybir.dt.float32
BF16 = mybir.dt.bfloat16
FP16 = mybir.dt.float16
AX = mybir.AxisListType
ALU = mybir.AluOpType
ACTF = mybir.ActivationFunctionType

N_CORES = 8
B = 256
S = B // N_CORES          # samples per core
C_IN, H, W = 3, 64, 64
N_EXPERTS, TOP_K, C_OUT = 4, 2, 64
HP, WP = H + 2, W + 2     # 66x66 zero-padded frame
FRAME = 34                # padded 32x32 frame rows for conv2 input
XRF = H * W               # 4096: pre-shifted replica, contiguous 64x64

# tunables (engine assignment / scheme mix)
N_DSCHEME = 4             # of 8 conv1-expert blocks per sample use double-ACT drain

bf16 = ml_dtypes.bfloat16
DEBUG_DUMP = False


# --------------------------------------------------------------------------
# Walrus in this environment accepts at most ONE sync wait per instruction.
# Split extra waits onto same-engine NoOps inserted right before.
# --------------------------------------------------------------------------
def _legalize_single_wait(nc):
    for _name, bbb in nc.bb_map.items():
        bb = bbb.bb if hasattr(bbb, "bb") else bbb
        insts = bb.instructions
        i = 0
        while i < len(insts):
            inst = insts[i]
            si = inst.sync_info
            if si is not None and len(si.on_wait) > 1:
                waits = list(si.on_wait)
                si.on_wait = [waits[-1]]
                for w in waits[:-1]:
                    nop = bass_rust.InstNoOp(
                        name=nc.get_next_instruction_name(), engine=inst.engine
                    )
                    nop.sync_info = mybir.SyncInfo(on_wait=[w], on_update=[])
                    nc.register_instruction(nop)
                    insts.insert(i, nop)
                    i += 1
            i += 1


class SplitWaitTileContext(tile.TileContext):
    def _drain_and_barrier(self, tick_clock, wait_clock):
        drain_inst = self.nc.sync.drain()
        wait_clock.add_sem_waits(
            drain_inst.ins, ScopedClock({None: tick_clock.global_clock})
        )
        self.nc.all_engine_barrier()
        assert self.sems is not None
        popped = self.nc._tile_sem_poison_stack.pop()
        assert popped is self._sem_poison
        self.nc.clear_and_free_semaphores(list(self.sems.allocated().values()))
        self.nc.all_engine_barrier()
        _legalize_single_wait(self.nc)


# --------------------------------------------------------------------------
# Device program
# --------------------------------------------------------------------------
def _emit(nc, tc, ctx, s_per_core, loop_n=1):
    Sn = s_per_core
    xr_d = nc.dram_tensor("xr", [Sn, 82, XRF], BF16, kind="ExternalInput")
    w1_d = nc.dram_tensor("w1", [82, 192], BF16, kind="ExternalInput")
    w2_d = nc.dram_tensor("w2", [97, 4, 3, 64], BF16, kind="ExternalInput")
    efwp_d = nc.dram_tensor("efwp", [4, 128, 128, 128], BF16, kind="ExternalInput")
    efb_d = nc.dram_tensor("efb", [1, 4, 128], BF16, kind="ExternalInput")
    gfb_d = nc.dram_tensor("gfb", [1, 4], FP16, kind="ExternalInput")
    gfwp_d = nc.dram_tensor("gfwp", [128, 512, 4], FP16, kind="ExternalInput")
    fwt_d = nc.dram_tensor("fwt", [128, 10], F32, kind="ExternalInput")
    fb_d = nc.dram_tensor("fb", [10, 1], F32, kind="ExternalInput")
    out_d = nc.dram_tensor("out", [10, Sn], F32, kind="ExternalOutput")

    singles = ctx.enter_context(tc.tile_pool(name="singles", bufs=1))
    persist = ctx.enter_context(tc.tile_pool(name="persist", bufs=1))
    stage = ctx.enter_context(tc.tile_pool(name="stage", bufs=2))
    evpool = ctx.enter_context(tc.tile_pool(name="evpool", bufs=4))
    wpool = ctx.enter_context(tc.tile_pool(name="wpool", bufs=4))
    wmgpool = ctx.enter_context(tc.tile_pool(name="wmgpool", bufs=3))
    psE = ctx.enter_context(tc.tile_pool(name="psE", bufs=3, space="PSUM"))
    psf = ctx.enter_context(tc.tile_pool(name="psf", bufs=1, space="PSUM"))
    prepool = ctx.enter_context(tc.tile_pool(name="prepool", bufs=4))
    efpool = ctx.enter_context(tc.tile_pool(name="efpool", bufs=3))

    # ---- weights / constants to SBUF ----
    w1_sb = singles.tile([82, 192], BF16)
    nc.sync.dma_start(out=w1_sb, in_=w1_d[:])
    w2_sb = singles.tile([97, 4, 3, 64], BF16)
    nc.sync.dma_start(out=w2_sb, in_=w2_d[:])
    gfwp_sb = singles.tile([128, 512, 4], FP16)
    nc.sync.dma_start(out=gfwp_sb, in_=gfwp_d[:])
    fwt_sb = singles.tile([128, 10], F32)
    nc.sync.dma_start(out=fwt_sb, in_=fwt_d[:])
    fb_sb = singles.tile([10, 1], F32)
    nc.sync.dma_start(out=fb_sb, in_=fb_d[:])
    efb_sb = singles.tile([1, 4, 128], BF16)
    nc.sync.dma_start(out=efb_sb, in_=efb_d[:])
    gfb_sb = singles.tile([1, 4], FP16)
    nc.sync.dma_start(out=gfb_sb, in_=gfb_d[:])
    ident = singles.tile([128, 128], F32)
    make_identity(nc, ident)
    ones_bf = singles.tile([1, Sn], BF16)
    nc.vector.memset(ones_bf, 1.0)
    ones_h = singles.tile([1, Sn], FP16)
    nc.vector.memset(ones_h, 1.0)
    ones_f = singles.tile([1, Sn], F32)
    nc.vector.memset(ones_f, 1.0)

    # ---- persistent activations ----
    # gpoolP[(jbit,ch), s, j]: pooled gate features, j = b*128+hp*32+wp (b<4)
    gpoolP = persist.tile([128, Sn, 512], FP16, tag="gpoolP")
    # h2P[(jbit,o2), e, s, j]: pooled conv2 features, j = (hp%8)*16+wp
    h2P = persist.tile([128, 4, Sn, 128], BF16, tag="h2P")
    # row 81 of xr is all-ones (host-provided): bias folded into w1 row 81
    xr_sb = [persist.tile([82, XRF], BF16, tag=f"xr{i}", name=f"xr{i}") for i in range(3)]
    # fr: conv1 expert pooled frames, partitions (o32, e4); zero borders once
    fr_sb = [persist.tile([128, FRAME * FRAME], BF16, tag=f"fr{i}", name=f"frames{i}") for i in range(2)]
    for fr in fr_sb:
        f34 = fr.rearrange("p (h w) -> p h w", h=FRAME)
        nc.vector.memset(f34[:, 0, :], 0.0)
        nc.vector.memset(f34[:, 33, :], 0.0)
        nc.vector.memset(f34[:, 1:33, 0], 0.0)
        nc.vector.memset(f34[:, 1:33, 33], 0.0)
    # F3[(dx,o32), e, flat]: dx-shifted replicas of fr for conv2 K=96 im2col
    # row 96 is all-ones: conv2 bias folded into w2 row 96 (dy=0 only)
    F3_sb = [persist.tile([97, 4, FRAME * FRAME], BF16, tag=f"F3{i}", name=f"F3{i}")
             for i in range(2)]
    for t in F3_sb:
        nc.vector.memset(t[96:97], 1.0)


    # ---- conv phase (optionally repeated on-device for timing) ----
    loop_cm = tc.For_i(0, loop_n, 1) if loop_n > 1 else None
    if loop_cm is not None:
        ctx.enter_context(loop_cm)

    pre_blocks = []   # prefetched expert-FC weight blocks (filled in conv tail)

    for s in range(Sn):
        xr = xr_sb[s % 3]
        fr = fr_sb[s % 2]
        if s == 0:
            nc.sync.dma_start(out=xr, in_=xr_d[0])
        xr66 = xr.rearrange("p (h w) -> p h w", h=H)
        f34 = fr.rearrange("p (h w) -> p h w", h=FRAME)

        # ---------- gate block: 2-bank unit, 2 halves x 2 col-tiled samples ----
        def emit_gate_block(gb):
            xr_prev = xr_sb[(s - 1) % 3]
            xr66p = xr_prev.rearrange("p (h w) -> p h w", h=H)
            pg = psE.tile([128, 2, 8, 64], F32, tag="psE", name=f"pg{s}_{gb}")
            for half in range(2):
                h0g = (gb * 2 + half) * 8
                nc.tensor.matmul(pg[0:64, half], w1_sb[:, 0:64],
                                 xr66p[:, h0g : h0g + 8, :],
                                 start=True, stop=True, tile_position=(0, 0))
                nc.tensor.matmul(pg[64:128, half], w1_sb[:, 0:64],
                                 xr66[:, h0g : h0g + 8, :],
                                 start=True, stop=True, tile_position=(0, 64))
            evg = evpool.tile([128, 2, 8, 32], F32, tag="ev")
            nc.scalar.activation(evg, pg[:, :, :, 0:64:2], ACTF.Relu)
            wmg = wmgpool.tile([128, 16, 32], FP16, tag="wmg")
            nc.vector.tensor_tensor(
                out=wmg.rearrange("p (a b) c -> p a b c", a=2),
                in0=pg[:, :, :, 1:64:2], in1=evg, op=ALU.max)
            jbit = gb // 2
            jcol = (gb % 2) * 256
            for sh in range(2):
                samp = s - 1 + sh
                dst = gpoolP[64 * jbit : 64 * jbit + 64, samp,
                             jcol : jcol + 256].rearrange(
                                 "p (a b) -> p a b", a=8)
                nc.vector.tensor_tensor(
                    out=dst,
                    in0=wmg[64 * sh : 64 * sh + 64, 0:16:2, :],
                    in1=wmg[64 * sh : 64 * sh + 64, 1:16:2, :],
                    op=ALU.max)

        # ---------- conv1 expert: 4 two-bank units ----------
        for pb2 in range(4):
            pe = psE.tile([128, 2, 8, 64], F32, tag="psE")
            for hh in range(2):
                h0 = (pb2 * 2 + hh) * 8
                nc.tensor.matmul(pe[:, hh], w1_sb[:, 64:192],
                                 xr66[:, h0 : h0 + 8, :], start=True, stop=True)
            ev = evpool.tile([128, 2, 8, 32], F32, tag="ev")
            nc.scalar.activation(ev, pe[:, :, :, 0:64:2], ACTF.Relu)
            wm1 = wpool.tile([128, 16, 32], BF16, tag="wm1")
            nc.vector.tensor_tensor(
                out=wm1.rearrange("p (a b) c -> p a b c", a=2),
                in0=pe[:, :, :, 1:64:2], in1=ev, op=ALU.max)
            # h-max into frame interior rows
            hp0 = pb2 * 8
            nc.vector.tensor_tensor(out=f34[:, 1 + hp0 : 9 + hp0, 1:33],
                                    in0=wm1[:, 0:16:2, :], in1=wm1[:, 1:16:2, :],
                                    op=ALU.max)
            if s % 2 == 1:
                emit_gate_block(pb2)

        # ---------- F3 build: 3 dx-shift replication DMAs ----------
        F3 = F3_sb[s % 2]
        # prefetch next sample's input on the free Sync queue
        if s + 1 < Sn:
            nc.sync.dma_start(out=xr_sb[(s + 1) % 3], in_=xr_d[s + 1])
        for g in range(3):
            nc.sync.dma_start(
                out=F3[32 * g : 32 * g + 32, :, 0 : FRAME * FRAME - g],
                in_=fr[:, g : FRAME * FRAME],
            )
        F3v = F3.rearrange("p e (h w) -> p e h w", h=FRAME)

        # ---------- conv2: per expert pair, one 2-bank psum tile ----------
        for pair in range(2):
            p2 = psE.tile([128, 2, 16, 32], F32, tag="psE")
            for dy in range(3):
                for side in range(2):
                    e = 2 * pair + side
                    for rh in range(2):
                        rhs = F3v[:, e, dy + rh * 16 : dy + rh * 16 + 16, 0:32]
                        nc.tensor.matmul(
                            p2[64 * side : 64 * side + 64, rh],
                            w2_sb[:, e, dy, :], rhs,
                            start=(dy == 0), stop=(dy == 2),
                            tile_position=(0, 64 * side),
                        )
            ev2 = evpool.tile([128, 2, 16, 16], F32, tag="ev")
            nc.scalar.activation(ev2, p2[:, :, :, 0:32:2], ACTF.Relu)
            wm2 = wpool.tile([128, 32, 16], BF16, tag="wm2")
            nc.vector.tensor_tensor(
                out=wm2.rearrange("p (a b) c -> p a b c", a=2),
                in0=p2[:, :, :, 1:32:2], in1=ev2, op=ALU.max)
            # h-max + remap into h2P (gpsimd: all bf16 sbuf)
            for side in range(2):
                e = 2 * pair + side
                for jbit in range(2):
                    dst = h2P[64 * jbit : 64 * jbit + 64, e, s, :].rearrange(
                        "p (a b) -> p a b", a=8)
                    nc.vector.tensor_tensor(
                        out=dst,
                        in0=wm2[64 * side : 64 * side + 64,
                                16 * jbit : 16 * jbit + 16 : 2, :],
                        in1=wm2[64 * side : 64 * side + 64,
                                16 * jbit + 1 : 16 * jbit + 16 : 2, :],
                        op=ALU.max)

        # ---------- expert-FC weight prefetch in the conv tail ----------
        if s >= Sn - 8 and s % 2 == 0:
            idx = (s - (Sn - 8)) // 2
            e, jb = divmod(idx, 4)
            blk = prepool.tile([128, 32, 128], BF16, tag="efblk",
                               name=f"pre{idx}")
            eng = nc.sync if idx % 2 == 0 else nc.scalar
            eng.dma_start(out=blk, in_=efwp_d[e, :, jb * 32 : jb * 32 + 32, :])
            pre_blocks.append(blk)

    # ---- FC phase ----
    # gate FC: col-tiled x4, fp32
    psG4 = psf.tile([128, 4], F32, tag="acc", name="psG4")
    for j in range(512):
        g = j % 4
        nc.tensor.matmul(psG4[32 * g : 32 * g + 32, :],
                         gpoolP[:, :, j], gfwp_sb[:, j, :],
                         start=(j < 4), stop=(j >= 508 and g != 0),
                         tile_position=(0, 32 * g))
    nc.tensor.matmul(psG4[0:32, :], ones_h, gfb_sb,
                     start=False, stop=True, tile_position=(0, 0))
    gacc = stage.tile([Sn, 2, 4], F32, tag="gacc", name="gacc")
    nc.scalar.activation(gacc[:, 0], psG4[32:64], ACTF.Identity)
    nc.scalar.activation(gacc[:, 1], psG4[96:128], ACTF.Identity)
    g1 = stage.tile([Sn, 4], F32, tag="g1")
    nc.vector.tensor_tensor(out=g1, in0=psG4[0:32], in1=gacc[:, 0], op=ALU.add)
    g2 = stage.tile([Sn, 4], F32, tag="g2")
    nc.vector.tensor_tensor(out=g2, in0=psG4[64:96], in1=gacc[:, 1], op=ALU.add)
    gs = stage.tile([Sn, 4], F32, tag="gs")
    nc.vector.tensor_tensor(out=gs, in0=g1, in1=g2, op=ALU.add)

    # expert FC: col-tiled x4; psF4[(g,s), o] accumulates group-g partials
    feats = persist.tile([Sn, 4, 128], F32, tag="feats")
    acc_sb = stage.tile([Sn, 2, 128], F32, tag="acc_sb", name="acc_sb")
    for e in range(4):
        psF4 = psf.tile([128, 128], F32, tag="acc", name=f"psF4_{e}")
        for jb in range(4):
            idx = e * 4 + jb
            if idx < len(pre_blocks):
                blk = pre_blocks[idx]
            else:
                blk = efpool.tile([128, 32, 128], BF16, tag="efblk",
                                  name=f"ef{e}_{jb}")
                eng = nc.sync if idx % 2 == 0 else nc.scalar
                eng.dma_start(out=blk, in_=efwp_d[e, :, jb * 32 : jb * 32 + 32, :])
            for jg in range(8):
                for g in range(4):
                    j = jb * 32 + jg * 4 + g
                    nc.tensor.matmul(psF4[32 * g : 32 * g + 32, :],
                                     h2P[:, e, :, j], blk[:, jg * 4 + g, :],
                                     start=(jb == 0 and jg == 0),
                                     stop=(jb == 3 and jg == 7 and g != 0),
                                     tile_position=(0, 32 * g))
        nc.tensor.matmul(psF4[0:32, :], ones_bf, efb_sb[:, e, :],
                         start=False, stop=True, tile_position=(0, 0))
        # reduce the 4 col-groups: ACT copies 2 groups to SBUF, DVE adds
        nc.scalar.activation(acc_sb[:, 0], psF4[32:64], ACTF.Identity)
        nc.scalar.activation(acc_sb[:, 1], psF4[96:128], ACTF.Identity)
        a1 = stage.tile([Sn, 128], F32, tag="a1")
        nc.vector.tensor_tensor(out=a1, in0=psF4[0:32], in1=acc_sb[:, 0],
                                op=ALU.add)
        a2 = stage.tile([Sn, 128], F32, tag="a2")
        nc.vector.tensor_tensor(out=a2, in0=psF4[64:96], in1=acc_sb[:, 1],
                                op=ALU.add)
        nc.vector.tensor_tensor(out=feats[:, e], in0=a1, in1=a2, op=ALU.add)

    # ---- softmax / top-2 / combine / head ----
    m01 = stage.tile([Sn, 1], F32, tag="m01")
    nc.vector.tensor_tensor(out=m01, in0=gs[:, 0:1], in1=gs[:, 1:2], op=ALU.max)
    m23 = stage.tile([Sn, 1], F32, tag="m23")
    nc.vector.tensor_tensor(out=m23, in0=gs[:, 2:3], in1=gs[:, 3:4], op=ALU.max)
    mx = stage.tile([Sn, 1], F32, tag="mx")
    nc.vector.tensor_tensor(out=mx, in0=m01, in1=m23, op=ALU.max)
    nmx = stage.tile([Sn, 1], F32, tag="nmx")
    nc.scalar.mul(nmx, mx, -1.0)
    ex = stage.tile([Sn, 4], F32, tag="ex")
    nc.scalar.activation(ex, gs, ACTF.Exp, bias=nmx)
    cnt = stage.tile([Sn, 4], F32, tag="cnt")
    nc.vector.memset(cnt, 0.0)
    tgt = stage.tile([Sn, 4], F32, tag="tgt")
    for ep in range(4):
        col = gs[:, ep : ep + 1].broadcast_to((Sn, 4))
        nc.vector.tensor_tensor(out=tgt, in0=col, in1=gs, op=ALU.is_gt)
        nc.vector.tensor_tensor(out=cnt, in0=cnt, in1=tgt, op=ALU.add)
        if ep < 3:
            ncols = 4 - (ep + 1)
            coleq = gs[:, ep : ep + 1].broadcast_to((Sn, ncols))
            teq = stage.tile([Sn, 4], F32, tag="teq")
            nc.vector.tensor_tensor(out=teq[:, : ncols], in0=coleq,
                                    in1=gs[:, ep + 1 :], op=ALU.is_equal)
            nc.vector.tensor_tensor(out=cnt[:, ep + 1 :], in0=cnt[:, ep + 1 :],
                                    in1=teq[:, : ncols], op=ALU.add)
    mask = stage.tile([Sn, 4], F32, tag="mask")
    nc.vector.tensor_scalar(out=mask, in0=cnt, scalar1=float(TOP_K), scalar2=None,
                            op0=ALU.is_lt)
    wm = stage.tile([Sn, 4], F32, tag="wm")
    nc.vector.tensor_tensor(out=wm, in0=ex, in1=mask, op=ALU.mult)
    s01 = stage.tile([Sn, 1], F32, tag="s01")
    nc.vector.tensor_tensor(out=s01, in0=wm[:, 0:1], in1=wm[:, 1:2], op=ALU.add)
    s23 = stage.tile([Sn, 1], F32, tag="s23")
    nc.vector.tensor_tensor(out=s23, in0=wm[:, 2:3], in1=wm[:, 3:4], op=ALU.add)
    sw = stage.tile([Sn, 1], F32, tag="sw")
    nc.vector.tensor_tensor(out=sw, in0=s01, in1=s23, op=ALU.add)
    rw = stage.tile([Sn, 1], F32, tag="rw")
    nc.vector.reciprocal(rw, sw)
    wn = stage.tile([Sn, 4], F32, tag="wn")
    nc.vector.tensor_scalar(out=wn, in0=wm, scalar1=rw, scalar2=None, op0=ALU.mult)

    feat = stage.tile([Sn, 128], F32, tag="feat")
    nc.vector.tensor_scalar(out=feat, in0=feats[:, 0], scalar1=wn[:, 0:1],
                            scalar2=None, op0=ALU.mult)
    for e in range(1, 4):
        nc.vector.scalar_tensor_tensor(out=feat, in0=feats[:, e],
                                       scalar=wn[:, e : e + 1], in1=feat,
                                       op0=ALU.mult, op1=ALU.add)
    # head: out.T = fw @ feat.T   (bias per-partition on the 10 outputs)
    psT = psf.tile([128, Sn], F32, tag="acc", name="psT")
    nc.tensor.transpose(psT, feat, ident[0:Sn, 0:Sn])
    featT = stage.tile([128, Sn], F32, tag="featT")
    nc.vector.tensor_copy(out=featT, in_=psT)
    psH = psf.tile([10, Sn], F32, tag="acc", name="psH")
    nc.tensor.matmul(psH, fwt_sb, featT, start=True, stop=True)
    outT = stage.tile([10, Sn], F32, tag="outT")
    nc.scalar.activation(outT, psH, ACTF.Identity, bias=fb_sb)
    nc.sync.dma_start(out=out_d[:], in_=outT)
    if DEBUG_DUMP:
        dbgw_d = nc.dram_tensor("dbgw", [Sn, 4], F32, kind="ExternalOutput")
        nc.sync.dma_start(out=dbgw_d[:], in_=wn)
        dbgft_d = nc.dram_tensor("dbgft", [Sn, 128], F32, kind="ExternalOutput")
        nc.sync.dma_start(out=dbgft_d[:], in_=feat)
        dbgs_d = nc.dram_tensor("dbgs", [Sn, 4], F32, kind="ExternalOutput")
        nc.sync.dma_start(out=dbgs_d[:], in_=gs)
        dbgf_d = nc.dram_tensor("dbgf", [Sn, 4, 128], F32, kind="ExternalOutput")
        nc.sync.dma_start(out=dbgf_d[:], in_=feats)
        dbgg_d = nc.dram_tensor("dbgg", [128, Sn, 512], BF16, kind="ExternalOutput")
        nc.sync.dma_start(out=dbgg_d[:], in_=gpoolP)
        dbgh_d = nc.dram_tensor("dbgh", [128, 4, Sn, 128], BF16, kind="ExternalOutput")
        nc.sync.dma_start(out=dbgh_d[:], in_=h2P)


def build_nc(s_per_core=S, loop_n=1):
    nc = bass.Bass()
    with ExitStack() as ctx:
        tc = ctx.enter_context(SplitWaitTileContext(nc))
        _emit(nc, tc, ctx, s_per_core, loop_n=loop_n)
    return nc


# --------------------------------------------------------------------------
# Host-side input preparation
# --------------------------------------------------------------------------
def _split_hi_lo(a):
    hi = a.astype(bf16)
    lo = (a - hi.astype(np.float32)).astype(bf16)
    return hi, lo


def prep_inputs(x, ew1, eb1, ew2, eb2, efw, efb, gw, gb, gfw, gfb, fw, fb):
    # x -> padded 66x66 frames, then 81-row pre-shifted 64x64 im2col replicas
    # (each replica row stores its own (dy,dx)-shifted window -> contiguous rhs)
    xp = np.zeros((B, C_IN, HP, HP), np.float32)
    xp[:, :, 1:65, 1:65] = x
    x_hi, x_lo = _split_hi_lo(xp)
    xr = np.empty((B, 82, XRF), bf16)
    r = 0
    for srca in (x_hi, x_hi, x_lo):
        for c in range(C_IN):
            for dy in range(3):
                for dx in range(3):
                    xr[:, r] = srca[:, c, dy : dy + H, dx : dx + W].reshape(B, XRF)
                    r += 1
    assert r == 81
    xr[:, 81] = bf16(1.0)

    # conv1 fused weights: [82, 192] = 3 K-terms x (64 gate | 128 expert(o,e))
    # + row 81 = bias (applied via the all-ones xr row 81)
    wg = gw.transpose(1, 2, 3, 0).reshape(27, 64)            # (c,dy,dx) x o
    we = ew1.transpose(2, 3, 4, 1, 0).reshape(27, 128)       # (c,dy,dx) x (o,e)
    w_full = np.concatenate([wg, we], axis=1).astype(np.float32)
    w_hi, w_lo = _split_hi_lo(w_full)
    bias1 = np.concatenate([gb, eb1.T.reshape(128)]).reshape(1, 192)
    w1 = np.concatenate([w_hi, w_lo, w_hi, bias1.astype(np.float32)], axis=0)

    # conv2 K=96 im2col weights: w2[g*32+i, e, dy, o] = ew2[e, o, i, dy, g]
    # + row 96 = bias on dy=0 (applied via the all-ones F3 row 96)
    w2 = np.zeros((97, 4, 3, 64), np.float32)
    w2[:96] = ew2.transpose(4, 2, 0, 3, 1).reshape(96, 4, 3, 64)
    w2[96, :, 0, :] = eb2                                    # [e, o]
    w2 = np.ascontiguousarray(w2.astype(bf16))

    # expert FC weights: h2P[(jbit, o2), e, s, j] = pooled[o2, 8*jbit + j//16,
    # j%16]  ->  flat = o2*256 + jbit*128 + j
    efwp = np.ascontiguousarray(
        efw.reshape(4, 128, 64, 2, 128).transpose(0, 3, 2, 4, 1)
        .reshape(4, 128, 128, 128).astype(bf16))

    # gate FC weights: gpoolP[(jbit, ch), s, jcol] -> flat = ch*1024 + jbit*512 + jcol
    gfwp = np.ascontiguousarray(
        gfw.reshape(4, 64, 2, 512).transpose(2, 1, 3, 0)
        .reshape(128, 512, 4).astype(np.float16))

    fwt = fw.T.astype(np.float32).copy()                     # [128, 10]
    fbv = fb.reshape(10, 1).astype(np.float32)

    shared = {
        "w1": np.ascontiguousarray(w1.astype(bf16)),
        "w2": np.ascontiguousarray(w2),
        "efwp": efwp, "gfwp": gfwp, "fwt": fwt, "fb": fbv,
        "efb": np.ascontiguousarray(efb.reshape(1, 4, 128).astype(bf16)),
        "gfb": np.ascontiguousarray(gfb.reshape(1, 4).astype(np.float16)),
    }
    return xr, shared


# --------------------------------------------------------------------------
# Persistent runner (same as baseline)
# --------------------------------------------------------------------------
class _Runner:
    def __init__(self, nc, n_cores):
        import jax
        from jax.experimental.shard_map import shard_map
        from jax.sharding import Mesh, PartitionSpec, NamedSharding
        from concourse import bass2jax

        bass2jax.install_neuronx_cc_hook()
        self.jax = jax
        self.nc = nc
        self.n_cores = n_cores
        partition_name = (
            nc.partition_id_tensor.name if nc.partition_id_tensor else None
        )
        in_names, out_names, out_avals, zero_outs = [], [], [], []
        for alloc in nc.m.functions[0].allocations:
            if not isinstance(alloc, mybir.MemoryLocationSet):
                continue
            name = alloc.memorylocations[0].name
            if alloc.kind == "ExternalInput":
                if name == partition_name:
                    continue
                in_names.append(name)
            elif alloc.kind == "ExternalOutput":
                out_names.append(name)
                shape = tuple(alloc.tensor_shape)
                dtype = mybir.dt.np(alloc.dtype)
                out_avals.append(jax.core.ShapedArray(shape, dtype))
                zero_outs.append(np.zeros(shape, dtype))
        self.in_names = list(in_names)
        self.out_names = out_names
        self.zero_outs = zero_outs
        n_params = len(in_names)
        all_names = in_names + out_names
        if partition_name is not None:
            all_names = all_names + [partition_name]
        donate = tuple(range(n_params, n_params + len(out_names)))
        out_avals_t = tuple(out_avals)

        def _body(*args):
            operands = list(args)
            if partition_name is not None:
                operands.append(bass2jax.partition_id_tensor())
            outs = bass2jax._bass_exec_p.bind(
                *operands,
                out_avals=out_avals_t,
                in_names=tuple(all_names),
                out_names=tuple(out_names),
                lowering_input_output_aliases=(),
                sim_require_finite=True,
                sim_require_nnan=True,
                nc=nc,
            )
            return tuple(outs)

        devices = jax.devices()[:n_cores]
        self.mesh = Mesh(np.asarray(devices), ("core",))
        self.sharding = NamedSharding(self.mesh, PartitionSpec("core"))
        in_specs = (PartitionSpec("core"),) * (n_params + len(out_names))
        out_specs = (PartitionSpec("core"),) * len(out_names)
        self.fn = jax.jit(
            shard_map(_body, mesh=self.mesh, in_specs=in_specs,
                      out_specs=out_specs, check_rep=False),
            donate_argnums=donate, keep_unused=True,
        )

    def concat_inputs(self, in_maps):
        return [
            np.concatenate([np.asarray(m[name]) for m in in_maps], axis=0)
            for name in self.in_names
        ]

    def put(self, concat_in):
        return [self.jax.device_put(a, self.sharding) for a in concat_in]

    def call(self, device_in):
        zeros = [np.zeros((self.n_cores * z.shape[0], *z.shape[1:]), z.dtype)
                 for z in self.zero_outs]
        outs = self.fn(*device_in, *zeros)
        return outs

    def run(self, in_maps):
        outs = self.call(self.put(self.concat_inputs(in_maps)))
        n = self.n_cores
        res = []
        for c in range(n):
            d = {}
            for i, name in enumerate(self.out_names):
                full = np.asarray(outs[i])
                d[name] = full.reshape(n, full.shape[0] // n, *full.shape[1:])[c]
            res.append(d)
        return res


_RUNNER = None


def get_runner():
    global _RUNNER
    if _RUNNER is None:
        _RUNNER = _Runner(build_nc(S), N_CORES)
    return _RUNNER


def kernel(**inputs):
    xr, shared = prep_inputs(**{k: np.asarray(v) for k, v in inputs.items()})
    runner = get_runner()
    in_maps = []
    for c in range(N_CORES):
        m = dict(shared)
        m["xr"] = np.ascontiguousarray(xr[c * S : (c + 1) * S])
        in_maps.append(m)
    res = runner.run(in_maps)
    out = np.empty((B, 10), np.float32)
    for c in range(N_CORES):
        out[c * S : (c + 1) * S] = res[c]["out"].T
    return out



# revision 34
# speedup vs baseline: 1.8531x; 1.0255x over previous
"""Trainium2 Bass kernel for nn_CNNFeatMoe (CNN feature MoE with top-2 routing).

V1 redesign vs baseline:
  - pooling pipeline rebalanced across ACT/DVE/Pool(gpsimd):
      ACT drains psum-even (relu+bias), DVE psum-odd (stt bias+max),
      h-max second stage on Pool/DVE (tunable), some conv1 tiles use
      double-ACT drain + DVE-4x sbuf w-max ("D-scheme") to offload DVE.
  - gate conv1 pair-packed: 2 samples per psum tile via PE column tiling.
  - conv2 im2col frames built as ONE tile F3 [96=(dx,ch), 4=expert, 34*34]
    -> 3 SBUF DMAs per sample instead of 12 (conv1 expert weight columns
    reordered host-side to (ch, expert)).
  - pooled gate/conv2 outputs written DIRECTLY into the FC-layout tensors
    (gpoolP / h2P) by the h-max ops; gfw/efw reordered host-side to match.
  - FC phase: PE column-tiling x4 (4 concurrent K-chunks), deep weight
    prefetch on two HWDGE queues (SP + ACT).
  - softmax tail: pairwise max/sum instead of tensor_reduce.
"""

import numpy as np
import ml_dtypes
from contextlib import ExitStack

import concourse.bass as bass
import concourse.mybir as mybir
import concourse.tile as tile
from concourse.vector_clock import ScopedClock
from concourse.masks import make_identity
import bass_rust

F32 = mybir.dt.float32
BF16 = mybir.dt.bfloat16
FP16 = mybir.dt.float16
FP16 = mybir.dt.float16
AX = mybir.AxisListType
ALU = mybir.AluOpType
ACTF = mybir.ActivationFunctionType

N_CORES = 8
B = 256
S = B // N_CORES          # samples per core
C_IN, H, W = 3, 64, 64
N_EXPERTS, TOP_K, C_OUT = 4, 2, 64
HP, WP = H + 2, W + 2     # 66x66 zero-padded frame
FRAME = 34                # padded 32x32 frame rows for conv2 input
XRF = H * W               # 4096: pre-shifted replica, contiguous 64x64

# tunables (engine assignment / scheme mix)
N_DSCHEME = 4             # of 8 conv1-expert blocks per sample use double-ACT drain

bf16 = ml_dtypes.bfloat16
DEBUG_DUMP = False


# --------------------------------------------------------------------------
# Walrus in this environment accepts at most ONE sync wait per instruction.
# Split extra waits onto same-engine NoOps inserted right before.
# --------------------------------------------------------------------------
def _legalize_single_wait(nc):
    for _name, bbb in nc.bb_map.items():
        bb = bbb.bb if hasattr(bbb, "bb") else bbb
        insts = bb.instructions
        i = 0
        while i < len(insts):
            inst = insts[i]
            si = inst.sync_info
            if si is not None and len(si.on_wait) > 1:
                waits = list(si.on_wait)
                si.on_wait = [waits[-1]]
                for w in waits[:-1]:
                    nop = bass_rust.InstNoOp(
                        name=nc.get_next_instruction_name(), engine=inst.engine
                    )
                    nop.sync_info = mybir.SyncInfo(on_wait=[w], on_update=[])
                    nc.register_instruction(nop)
                    insts.insert(i, nop)
                    i += 1
            i += 1


class SplitWaitTileContext(tile.TileContext):
    def _drain_and_barrier(self, tick_clock, wait_clock):
        drain_inst = self.nc.sync.drain()
        wait_clock.add_sem_waits(
            drain_inst.ins, ScopedClock({None: tick_clock.global_clock})
        )
        self.nc.all_engine_barrier()
        assert self.sems is not None
        popped = self.nc._tile_sem_poison_stack.pop()
        assert popped is self._sem_poison
        self.nc.clear_and_free_semaphores(list(self.sems.allocated().values()))
        self.nc.all_engine_barrier()
        _legalize_single_wait(self.nc)


# --------------------------------------------------------------------------
# Device program
# --------------------------------------------------------------------------
def _emit(nc, tc, ctx, s_per_core, loop_n=1):
    Sn = s_per_core
    xr_d = nc.dram_tensor("xr", [Sn, 82, XRF], BF16, kind="ExternalInput")
    w1_d = nc.dram_tensor("w1", [82, 192], BF16, kind="ExternalInput")
    w2_d = nc.dram_tensor("w2", [97, 4, 3, 64], BF16, kind="ExternalInput")
    efwp_d = nc.dram_tensor("efwp", [4, 128, 128, 128], BF16, kind="ExternalInput")
    efb_d = nc.dram_tensor("efb", [1, 4, 128], BF16, kind="ExternalInput")
    gfb_d = nc.dram_tensor("gfb", [1, 4], FP16, kind="ExternalInput")
    gfwp_d = nc.dram_tensor("gfwp", [128, 512, 4], FP16, kind="ExternalInput")
    fwt_d = nc.dram_tensor("fwt", [128, 10], F32, kind="ExternalInput")
    fb_d = nc.dram_tensor("fb", [10, 1], F32, kind="ExternalInput")
    out_d = nc.dram_tensor("out", [10, Sn], F32, kind="ExternalOutput")

    singles = ctx.enter_context(tc.tile_pool(name="singles", bufs=1))
    persist = ctx.enter_context(tc.tile_pool(name="persist", bufs=1))
    stage = ctx.enter_context(tc.tile_pool(name="stage", bufs=2))
    evpool = ctx.enter_context(tc.tile_pool(name="evpool", bufs=4))
    wpool = ctx.enter_context(tc.tile_pool(name="wpool", bufs=4))
    wmgpool = ctx.enter_context(tc.tile_pool(name="wmgpool", bufs=3))
    psE = ctx.enter_context(tc.tile_pool(name="psE", bufs=3, space="PSUM"))
    psf = ctx.enter_context(tc.tile_pool(name="psf", bufs=1, space="PSUM"))
    prepool = ctx.enter_context(tc.tile_pool(name="prepool", bufs=4))
    efpool = ctx.enter_context(tc.tile_pool(name="efpool", bufs=3))
    prepool = ctx.enter_context(tc.tile_pool(name="prepool", bufs=4))

    # ---- weights / constants to SBUF ----
    w1_sb = singles.tile([82, 192], BF16)
    nc.sync.dma_start(out=w1_sb, in_=w1_d[:])
    w2_sb = singles.tile([97, 4, 3, 64], BF16)
    nc.sync.dma_start(out=w2_sb, in_=w2_d[:])
    gfwp_sb = singles.tile([128, 512, 4], FP16)
    nc.sync.dma_start(out=gfwp_sb, in_=gfwp_d[:])
    fwt_sb = singles.tile([128, 10], F32)
    nc.sync.dma_start(out=fwt_sb, in_=fwt_d[:])
    fb_sb = singles.tile([10, 1], F32)
    nc.sync.dma_start(out=fb_sb, in_=fb_d[:])
    efb_sb = singles.tile([1, 4, 128], BF16)
    nc.sync.dma_start(out=efb_sb, in_=efb_d[:])
    gfb_sb = singles.tile([1, 4], FP16)
    nc.sync.dma_start(out=gfb_sb, in_=gfb_d[:])
    ident = singles.tile([128, 128], F32)
    make_identity(nc, ident)
    ones_bf = singles.tile([1, Sn], BF16)
    nc.vector.memset(ones_bf, 1.0)
    ones_h = singles.tile([1, Sn], FP16)
    nc.vector.memset(ones_h, 1.0)
    ones_f = singles.tile([1, Sn], F32)
    nc.vector.memset(ones_f, 1.0)

    # ---- persistent activations ----
    # gpoolP[(jbit,ch), s, j]: pooled gate features, j = b*128+hp*32+wp (b<4)
    gpoolP = persist.tile([128, Sn, 512], FP16, tag="gpoolP")
    # h2P[(jbit,o2), e, s, j]: pooled conv2 features, j = (hp%8)*16+wp
    h2P = persist.tile([128, 4, Sn, 128], BF16, tag="h2P")
    # row 81 of xr is all-ones (host-provided): bias folded into w1 row 81
    xr_sb = [persist.tile([82, XRF], BF16, tag=f"xr{i}", name=f"xr{i}") for i in range(3)]
    # fr: conv1 expert pooled frames, partitions (o32, e4); zero borders once
    fr_sb = [persist.tile([128, FRAME * FRAME], BF16, tag=f"fr{i}", name=f"frames{i}") for i in range(2)]
    for fr in fr_sb:
        f34 = fr.rearrange("p (h w) -> p h w", h=FRAME)
        nc.vector.memset(f34[:, 0, :], 0.0)
        nc.vector.memset(f34[:, 33, :], 0.0)
        nc.vector.memset(f34[:, 1:33, 0], 0.0)
        nc.vector.memset(f34[:, 1:33, 33], 0.0)
    # F3[(dx,o32), e, flat]: dx-shifted replicas of fr for conv2 K=96 im2col
    # row 96 is all-ones: conv2 bias folded into w2 row 96 (dy=0 only)
    F3_sb = [persist.tile([97, 4, FRAME * FRAME], BF16, tag=f"F3{i}", name=f"F3{i}")
             for i in range(2)]
    for t in F3_sb:
        nc.vector.memset(t[96:97], 1.0)


    # ---- conv phase (optionally repeated on-device for timing) ----
    loop_cm = tc.For_i(0, loop_n, 1) if loop_n > 1 else None
    if loop_cm is not None:
        ctx.enter_context(loop_cm)

    pre_blocks = []   # prefetched expert-FC weight blocks (filled in conv tail)

    for s in range(Sn):
        xr = xr_sb[s % 3]
        fr = fr_sb[s % 2]
        if s == 0:
            nc.sync.dma_start(out=xr, in_=xr_d[0])
        xr66 = xr.rearrange("p (h w) -> p h w", h=H)
        f34 = fr.rearrange("p (h w) -> p h w", h=FRAME)

        # ---------- gate block: 2-bank unit, 2 halves x 2 col-tiled samples ----
        def emit_gate_block(gb):
            xr_prev = xr_sb[(s - 1) % 3]
            xr66p = xr_prev.rearrange("p (h w) -> p h w", h=H)
            pg = psE.tile([128, 2, 8, 64], F32, tag="psE", name=f"pg{s}_{gb}")
            for half in range(2):
                h0g = (gb * 2 + half) * 8
                nc.tensor.matmul(pg[0:64, half], w1_sb[:, 0:64],
                                 xr66p[:, h0g : h0g + 8, :],
                                 start=True, stop=True, tile_position=(0, 0))
                nc.tensor.matmul(pg[64:128, half], w1_sb[:, 0:64],
                                 xr66[:, h0g : h0g + 8, :],
                                 start=True, stop=True, tile_position=(0, 64))
            evg = evpool.tile([128, 2, 8, 32], F32, tag="ev")
            nc.scalar.activation(evg, pg[:, :, :, 0:64:2], ACTF.Relu)
            wmg = wmgpool.tile([128, 16, 32], FP16, tag="wmg")
            nc.vector.tensor_tensor(
                out=wmg.rearrange("p (a b) c -> p a b c", a=2),
                in0=pg[:, :, :, 1:64:2], in1=evg, op=ALU.max)
            jbit = gb // 2
            jcol = (gb % 2) * 256
            for sh in range(2):
                samp = s - 1 + sh
                dst = gpoolP[64 * jbit : 64 * jbit + 64, samp,
                             jcol : jcol + 256].rearrange(
                                 "p (a b) -> p a b", a=8)
                nc.vector.tensor_tensor(
                    out=dst,
                    in0=wmg[64 * sh : 64 * sh + 64, 0:16:2, :],
                    in1=wmg[64 * sh : 64 * sh + 64, 1:16:2, :],
                    op=ALU.max)

        # ---------- conv1 expert: 4 two-bank units ----------
        for pb2 in range(4):
            pe = psE.tile([128, 2, 8, 64], F32, tag="psE")
            for hh in range(2):
                h0 = (pb2 * 2 + hh) * 8
                nc.tensor.matmul(pe[:, hh], w1_sb[:, 64:192],
                                 xr66[:, h0 : h0 + 8, :], start=True, stop=True)
            ev = evpool.tile([128, 2, 8, 32], F32, tag="ev")
            nc.scalar.activation(ev, pe[:, :, :, 0:64:2], ACTF.Relu)
            wm1 = wpool.tile([128, 16, 32], BF16, tag="wm1")
            nc.vector.tensor_tensor(
                out=wm1.rearrange("p (a b) c -> p a b c", a=2),
                in0=pe[:, :, :, 1:64:2], in1=ev, op=ALU.max)
            # h-max into frame interior rows
            hp0 = pb2 * 8
            nc.vector.tensor_tensor(out=f34[:, 1 + hp0 : 9 + hp0, 1:33],
                                    in0=wm1[:, 0:16:2, :], in1=wm1[:, 1:16:2, :],
                                    op=ALU.max)
            if s % 2 == 1:
                emit_gate_block(pb2)

        # ---------- F3 build: 3 dx-shift replication DMAs ----------
        F3 = F3_sb[s % 2]
        # prefetch next sample's input on the free Sync queue
        if s + 1 < Sn:
            nc.sync.dma_start(out=xr_sb[(s + 1) % 3], in_=xr_d[s + 1])
        for g in range(3):
            nc.sync.dma_start(
                out=F3[32 * g : 32 * g + 32, :, 0 : FRAME * FRAME - g],
                in_=fr[:, g : FRAME * FRAME],
            )
        F3v = F3.rearrange("p e (h w) -> p e h w", h=FRAME)

        # ---------- conv2: per expert pair, one 2-bank psum tile ----------
        for pair in range(2):
            p2 = psE.tile([128, 2, 16, 32], F32, tag="psE")
            for dy in range(3):
                for side in range(2):
                    e = 2 * pair + side
                    for rh in range(2):
                        rhs = F3v[:, e, dy + rh * 16 : dy + rh * 16 + 16, 0:32]
                        nc.tensor.matmul(
                            p2[64 * side : 64 * side + 64, rh],
                            w2_sb[:, e, dy, :], rhs,
                            start=(dy == 0), stop=(dy == 2),
                            tile_position=(0, 64 * side),
                        )
            ev2 = evpool.tile([128, 2, 16, 16], F32, tag="ev")
            nc.scalar.activation(ev2, p2[:, :, :, 0:32:2], ACTF.Relu)
            wm2 = wpool.tile([128, 32, 16], BF16, tag="wm2")
            nc.vector.tensor_tensor(
                out=wm2.rearrange("p (a b) c -> p a b c", a=2),
                in0=p2[:, :, :, 1:32:2], in1=ev2, op=ALU.max)
            # h-max + remap into h2P (gpsimd: all bf16 sbuf)
            for side in range(2):
                e = 2 * pair + side
                for jbit in range(2):
                    dst = h2P[64 * jbit : 64 * jbit + 64, e, s, :].rearrange(
                        "p (a b) -> p a b", a=8)
                    nc.vector.tensor_tensor(
                        out=dst,
                        in0=wm2[64 * side : 64 * side + 64,
                                16 * jbit : 16 * jbit + 16 : 2, :],
                        in1=wm2[64 * side : 64 * side + 64,
                                16 * jbit + 1 : 16 * jbit + 16 : 2, :],
                        op=ALU.max)

        # ---------- expert-FC weight prefetch in the conv tail ----------
        if s >= Sn - 8 and s % 2 == 0:
            idx = (s - (Sn - 8)) // 2
            e, jb = divmod(idx, 4)
            blk = prepool.tile([128, 32, 128], BF16, tag="efblk",
                               name=f"pre{idx}")
            eng = nc.sync if idx % 2 == 0 else nc.scalar
            eng.dma_start(out=blk, in_=efwp_d[e, :, jb * 32 : jb * 32 + 32, :])
            pre_blocks.append(blk)

        # expert-FC weight prefetch in the conv tail (hides FC DMA wait)
        if s >= Sn - 8 and s % 2 == 0:
            idx = (s - (Sn - 8)) // 2
            e_, jb_ = divmod(idx, 4)
            blk = prepool.tile([128, 32, 128], BF16, tag="preblk", name=f"pre{idx}")
            nc.scalar.dma_start(out=blk, in_=efwp_d[e_, :, jb_ * 32 : jb_ * 32 + 32, :])
            pre_blocks.append(blk)

    # ---- FC phase ----
    # gate FC: col-tiled x4, fp16 inputs
    psG4 = psf.tile([128, 4], F32, tag="acc", name="psG4")
    for j in range(512):
        g = j % 4
        nc.tensor.matmul(psG4[32 * g : 32 * g + 32, :],
                         gpoolP[:, :, j], gfwp_sb[:, j, :],
                         start=(j < 4), stop=(j >= 508 and g != 0),
                         tile_position=(0, 32 * g))
    nc.tensor.matmul(psG4[0:32, :], ones_h, gfb_sb,
                     start=False, stop=True, tile_position=(0, 0))
    gacc = stage.tile([Sn, 2, 4], F32, tag="gacc", name="gacc")
    nc.scalar.activation(gacc[:, 0], psG4[32:64], ACTF.Identity)
    nc.scalar.activation(gacc[:, 1], psG4[96:128], ACTF.Identity)
    g1 = stage.tile([Sn, 4], F32, tag="g1")
    nc.vector.tensor_tensor(out=g1, in0=psG4[0:32], in1=gacc[:, 0], op=ALU.add)
    g2 = stage.tile([Sn, 4], F32, tag="g2")
    nc.vector.tensor_tensor(out=g2, in0=psG4[64:96], in1=gacc[:, 1], op=ALU.add)
    gs = stage.tile([Sn, 4], F32, tag="gs")
    nc.vector.tensor_tensor(out=gs, in0=g1, in1=g2, op=ALU.add)

    # expert FC: col-tiled x4; psF4[(g,s), o] accumulates group-g partials
    feats = persist.tile([Sn, 4, 128], F32, tag="feats")
    acc_sb = stage.tile([Sn, 2, 128], F32, tag="acc_sb", name="acc_sb")
    for e in range(4):
        psF4 = psf.tile([128, 128], F32, tag="acc", name=f"psF4_{e}")
        for jb in range(4):
            idx = e * 4 + jb
            if idx < len(pre_blocks):
                blk = pre_blocks[idx]
            else:
                blk = efpool.tile([128, 32, 128], BF16, tag="efblk",
                                  name=f"ef{e}_{jb}")
                eng = nc.sync if idx % 2 == 0 else nc.scalar
                eng.dma_start(out=blk, in_=efwp_d[e, :, jb * 32 : jb * 32 + 32, :])
            for jg in range(8):
                for g in range(4):
                    j = jb * 32 + jg * 4 + g
                    nc.tensor.matmul(psF4[32 * g : 32 * g + 32, :],
                                     h2P[:, e, :, j], blk[:, jg * 4 + g, :],
                                     start=(jb == 0 and jg == 0),
                                     stop=(jb == 3 and jg == 7 and g != 0),
                                     tile_position=(0, 32 * g))
        nc.tensor.matmul(psF4[0:32, :], ones_bf, efb_sb[:, e, :],
                         start=False, stop=True, tile_position=(0, 0))
        # reduce the 4 col-groups: ACT copies 2 groups to SBUF, DVE adds
        nc.scalar.activation(acc_sb[:, 0], psF4[32:64], ACTF.Identity)
        nc.scalar.activation(acc_sb[:, 1], psF4[96:128], ACTF.Identity)
        a1 = stage.tile([Sn, 128], F32, tag="a1")
        nc.vector.tensor_tensor(out=a1, in0=psF4[0:32], in1=acc_sb[:, 0],
                                op=ALU.add)
        a2 = stage.tile([Sn, 128], F32, tag="a2")
        nc.vector.tensor_tensor(out=a2, in0=psF4[64:96], in1=acc_sb[:, 1],
                                op=ALU.add)
        nc.vector.tensor_tensor(out=feats[:, e], in0=a1, in1=a2, op=ALU.add)

    # ---- softmax / top-2 / combine / head ----
    m01 = stage.tile([Sn, 1], F32, tag="m01")
    nc.vector.tensor_tensor(out=m01, in0=gs[:, 0:1], in1=gs[:, 1:2], op=ALU.max)
    m23 = stage.tile([Sn, 1], F32, tag="m23")
    nc.vector.tensor_tensor(out=m23, in0=gs[:, 2:3], in1=gs[:, 3:4], op=ALU.max)
    mx = stage.tile([Sn, 1], F32, tag="mx")
    nc.vector.tensor_tensor(out=mx, in0=m01, in1=m23, op=ALU.max)
    nmx = stage.tile([Sn, 1], F32, tag="nmx")
    nc.scalar.mul(nmx, mx, -1.0)
    ex = stage.tile([Sn, 4], F32, tag="ex")
    nc.scalar.activation(ex, gs, ACTF.Exp, bias=nmx)
    cnt = stage.tile([Sn, 4], F32, tag="cnt")
    nc.vector.memset(cnt, 0.0)
    tgt = stage.tile([Sn, 4], F32, tag="tgt")
    for ep in range(4):
        col = gs[:, ep : ep + 1].broadcast_to((Sn, 4))
        nc.vector.tensor_tensor(out=tgt, in0=col, in1=gs, op=ALU.is_gt)
        nc.vector.tensor_tensor(out=cnt, in0=cnt, in1=tgt, op=ALU.add)
        if ep < 3:
            ncols = 4 - (ep + 1)
            coleq = gs[:, ep : ep + 1].broadcast_to((Sn, ncols))
            teq = stage.tile([Sn, 4], F32, tag="teq")
            nc.vector.tensor_tensor(out=teq[:, : ncols], in0=coleq,
                                    in1=gs[:, ep + 1 :], op=ALU.is_equal)
            nc.vector.tensor_tensor(out=cnt[:, ep + 1 :], in0=cnt[:, ep + 1 :],
                                    in1=teq[:, : ncols], op=ALU.add)
    mask = stage.tile([Sn, 4], F32, tag="mask")
    nc.vector.tensor_scalar(out=mask, in0=cnt, scalar1=float(TOP_K), scalar2=None,
                            op0=ALU.is_lt)
    wm = stage.tile([Sn, 4], F32, tag="wm")
    nc.vector.tensor_tensor(out=wm, in0=ex, in1=mask, op=ALU.mult)
    s01 = stage.tile([Sn, 1], F32, tag="s01")
    nc.vector.tensor_tensor(out=s01, in0=wm[:, 0:1], in1=wm[:, 1:2], op=ALU.add)
    s23 = stage.tile([Sn, 1], F32, tag="s23")
    nc.vector.tensor_tensor(out=s23, in0=wm[:, 2:3], in1=wm[:, 3:4], op=ALU.add)
    sw = stage.tile([Sn, 1], F32, tag="sw")
    nc.vector.tensor_tensor(out=sw, in0=s01, in1=s23, op=ALU.add)
    rw = stage.tile([Sn, 1], F32, tag="rw")
    nc.vector.reciprocal(rw, sw)
    wn = stage.tile([Sn, 4], F32, tag="wn")
    nc.vector.tensor_scalar(out=wn, in0=wm, scalar1=rw, scalar2=None, op0=ALU.mult)

    feat = stage.tile([Sn, 128], F32, tag="feat")
    nc.vector.tensor_scalar(out=feat, in0=feats[:, 0], scalar1=wn[:, 0:1],
                            scalar2=None, op0=ALU.mult)
    for e in range(1, 4):
        nc.vector.scalar_tensor_tensor(out=feat, in0=feats[:, e],
                                       scalar=wn[:, e : e + 1], in1=feat,
                                       op0=ALU.mult, op1=ALU.add)
    # head: out.T = fw @ feat.T   (bias per-partition on the 10 outputs)
    psT = psf.tile([128, Sn], F32, tag="acc", name="psT")
    nc.tensor.transpose(psT, feat, ident[0:Sn, 0:Sn])
    featT = stage.tile([128, Sn], F32, tag="featT")
    nc.vector.tensor_copy(out=featT, in_=psT)
    psH = psf.tile([10, Sn], F32, tag="acc", name="psH")
    nc.tensor.matmul(psH, fwt_sb, featT, start=True, stop=True)
    outT = stage.tile([10, Sn], F32, tag="outT")
    nc.scalar.activation(outT, psH, ACTF.Identity, bias=fb_sb)
    nc.sync.dma_start(out=out_d[:], in_=outT)
    if DEBUG_DUMP:
        dbgw_d = nc.dram_tensor("dbgw", [Sn, 4], F32, kind="ExternalOutput")
        nc.sync.dma_start(out=dbgw_d[:], in_=wn)
        dbgft_d = nc.dram_tensor("dbgft", [Sn, 128], F32, kind="ExternalOutput")
        nc.sync.dma_start(out=dbgft_d[:], in_=feat)
        dbgs_d = nc.dram_tensor("dbgs", [Sn, 4], F32, kind="ExternalOutput")
        nc.sync.dma_start(out=dbgs_d[:], in_=gs)
        dbgf_d = nc.dram_tensor("dbgf", [Sn, 4, 128], F32, kind="ExternalOutput")
        nc.sync.dma_start(out=dbgf_d[:], in_=feats)
        dbgg_d = nc.dram_tensor("dbgg", [128, Sn, 512], BF16, kind="ExternalOutput")
        nc.sync.dma_start(out=dbgg_d[:], in_=gpoolP)
        dbgh_d = nc.dram_tensor("dbgh", [128, 4, Sn, 128], BF16, kind="ExternalOutput")
        nc.sync.dma_start(out=dbgh_d[:], in_=h2P)


def build_nc(s_per_core=S, loop_n=1):
    nc = bass.Bass()
    with ExitStack() as ctx:
        tc = ctx.enter_context(SplitWaitTileContext(nc))
        _emit(nc, tc, ctx, s_per_core, loop_n=loop_n)
    return nc


# --------------------------------------------------------------------------
# Host-side input preparation
# --------------------------------------------------------------------------
def _split_hi_lo(a):
    hi = a.astype(bf16)
    lo = (a - hi.astype(np.float32)).astype(bf16)
    return hi, lo


def prep_inputs(x, ew1, eb1, ew2, eb2, efw, efb, gw, gb, gfw, gfb, fw, fb):
    # x -> padded 66x66 frames, then 81-row pre-shifted 64x64 im2col replicas
    # (each replica row stores its own (dy,dx)-shifted window -> contiguous rhs)
    xp = np.zeros((B, C_IN, HP, HP), np.float32)
    xp[:, :, 1:65, 1:65] = x
    x_hi, x_lo = _split_hi_lo(xp)
    xr = np.empty((B, 82, XRF), bf16)
    r = 0
    for srca in (x_hi, x_hi, x_lo):
        for c in range(C_IN):
            for dy in range(3):
                for dx in range(3):
                    xr[:, r] = srca[:, c, dy : dy + H, dx : dx + W].reshape(B, XRF)
                    r += 1
    assert r == 81
    xr[:, 81] = bf16(1.0)

    # conv1 fused weights: [82, 192] = 3 K-terms x (64 gate | 128 expert(o,e))
    # + row 81 = bias (applied via the all-ones xr row 81)
    wg = gw.transpose(1, 2, 3, 0).reshape(27, 64)            # (c,dy,dx) x o
    we = ew1.transpose(2, 3, 4, 1, 0).reshape(27, 128)       # (c,dy,dx) x (o,e)
    w_full = np.concatenate([wg, we], axis=1).astype(np.float32)
    w_hi, w_lo = _split_hi_lo(w_full)
    bias1 = np.concatenate([gb, eb1.T.reshape(128)]).reshape(1, 192)
    w1 = np.concatenate([w_hi, w_lo, w_hi, bias1.astype(np.float32)], axis=0)

    # conv2 K=96 im2col weights: w2[g*32+i, e, dy, o] = ew2[e, o, i, dy, g]
    # + row 96 = bias on dy=0 (applied via the all-ones F3 row 96)
    w2 = np.zeros((97, 4, 3, 64), np.float32)
    w2[:96] = ew2.transpose(4, 2, 0, 3, 1).reshape(96, 4, 3, 64)
    w2[96, :, 0, :] = eb2                                    # [e, o]
    w2 = np.ascontiguousarray(w2.astype(bf16))

    # expert FC weights: h2P[(jbit, o2), e, s, j] = pooled[o2, 8*jbit + j//16,
    # j%16]  ->  flat = o2*256 + jbit*128 + j
    efwp = np.ascontiguousarray(
        efw.reshape(4, 128, 64, 2, 128).transpose(0, 3, 2, 4, 1)
        .reshape(4, 128, 128, 128).astype(bf16))

    # gate FC weights: gpoolP[(jbit, ch), s, jcol] -> flat = ch*1024 + jbit*512 + jcol
    gfwp = np.ascontiguousarray(
        gfw.reshape(4, 64, 2, 512).transpose(2, 1, 3, 0)
        .reshape(128, 512, 4).astype(np.float16))

    fwt = fw.T.astype(np.float32).copy()                     # [128, 10]
    fbv = fb.reshape(10, 1).astype(np.float32)

    shared = {
        "w1": np.ascontiguousarray(w1.astype(bf16)),
        "w2": np.ascontiguousarray(w2),
        "efwp": efwp, "gfwp": gfwp, "fwt": fwt, "fb": fbv,
        "efb": np.ascontiguousarray(efb.reshape(1, 4, 128).astype(bf16)),
        "gfb": np.ascontiguousarray(gfb.reshape(1, 4).astype(np.float16)),
    }
    return xr, shared


# --------------------------------------------------------------------------
# Persistent runner (same as baseline)
# --------------------------------------------------------------------------
class _Runner:
    def __init__(self, nc, n_cores):
        import jax
        from jax.experimental.shard_map import shard_map
        from jax.sharding import Mesh, PartitionSpec, NamedSharding
        from concourse import bass2jax

        bass2jax.install_neuronx_cc_hook()
        self.jax = jax
        self.nc = nc
        self.n_cores = n_cores
        partition_name = (
            nc.partition_id_tensor.name if nc.partition_id_tensor else None
        )
        in_names, out_names, out_avals, zero_outs = [], [], [], []
        for alloc in nc.m.functions[0].allocations:
            if not isinstance(alloc, mybir.MemoryLocationSet):
                continue
            name = alloc.memorylocations[0].name
            if alloc.kind == "ExternalInput":
                if name == partition_name:
                    continue
                in_names.append(name)
            elif alloc.kind == "ExternalOutput":
                out_names.append(name)
                shape = tuple(alloc.tensor_shape)
                dtype = mybir.dt.np(alloc.dtype)
                out_avals.append(jax.core.ShapedArray(shape, dtype))
                zero_outs.append(np.zeros(shape, dtype))
        self.in_names = list(in_names)
        self.out_names = out_names
        self.zero_outs = zero_outs
        n_params = len(in_names)
        all_names = in_names + out_names
        if partition_name is not None:
            all_names = all_names + [partition_name]
        donate = tuple(range(n_params, n_params + len(out_names)))
        out_avals_t = tuple(out_avals)

        def _body(*args):
            operands = list(args)
            if partition_name is not None:
                operands.append(bass2jax.partition_id_tensor())
            outs = bass2jax._bass_exec_p.bind(
                *operands,
                out_avals=out_avals_t,
                in_names=tuple(all_names),
                out_names=tuple(out_names),
                lowering_input_output_aliases=(),
                sim_require_finite=True,
                sim_require_nnan=True,
                nc=nc,
            )
            return tuple(outs)

        devices = jax.devices()[:n_cores]
        self.mesh = Mesh(np.asarray(devices), ("core",))
        self.sharding = NamedSharding(self.mesh, PartitionSpec("core"))
        in_specs = (PartitionSpec("core"),) * (n_params + len(out_names))
        out_specs = (PartitionSpec("core"),) * len(out_names)
        self.fn = jax.jit(
            shard_map(_body, mesh=self.mesh, in_specs=in_specs,
                      out_specs=out_specs, check_rep=False),
            donate_argnums=donate, keep_unused=True,
        )

    def concat_inputs(self, in_maps):
        return [
            np.concatenate([np.asarray(m[name]) for m in in_maps], axis=0)
            for name in self.in_names
        ]

    def put(self, concat_in):
        return [self.jax.device_put(a, self.sharding) for a in concat_in]

    def call(self, device_in):
        zeros = [np.zeros((self.n_cores * z.shape[0], *z.shape[1:]), z.dtype)
                 for z in self.zero_outs]
        outs = self.fn(*device_in, *zeros)
        return outs

    def run(self, in_maps):
        outs = self.call(self.put(self.concat_inputs(in_maps)))
        n = self.n_cores
        res = []
        for c in range(n):
            d = {}
            for i, name in enumerate(self.out_names):
                full = np.asarray(outs[i])
                d[name] = full.reshape(n, full.shape[0] // n, *full.shape[1:])[c]
            res.append(d)
        return res


_RUNNER = None


def get_runner():
    global _RUNNER
    if _RUNNER is None:
        _RUNNER = _Runner(build_nc(S), N_CORES)
    return _RUNNER


def kernel(**inputs):
    xr, shared = prep_inputs(**{k: np.asarray(v) for k, v in inputs.items()})
    runner = get_runner()
    in_maps = []
    for c in range(N_CORES):
        m = dict(shared)
        m["xr"] = np.ascontiguousarray(xr[c * S : (c + 1) * S])
        in_maps.append(m)
    res = runner.run(in_maps)
    out = np.empty((B, 10), np.float32)
    for c in range(N_CORES):
        out[c * S : (c + 1) * S] = res[c]["out"].T
    return out

